# revision 55
# baseline (speedup 1.0000x reference)
"""Causal multi-head attention block (b=4, s=2048, d=1024, 16 heads) on 8
Trainium2 NeuronCores.

Sharding: tensor-parallel over heads x data-parallel over batch.
Core c handles batch c//2 and head-half c%2 (8 of 16 heads):
  - QKV projection for its 8 heads over all 2048 tokens (bf16 matmuls,
    fp32 PSUM accumulation)
  - causal attention in [k, q] score layout: scores for the even/odd head of
    a pair run concurrently in disjoint PE row-quadrants; softmax denominator
    comes for free from a ones-column appended to the V stationary; diagonal
    key-blocks restrict score/exp/PV work to the causally-valid query columns
    and multiply a single [128, 2x128] triangular mask on P' (DVE)
  - per-qc batched normalization: denominator rows staged to [8, 512], one
    Ln + one Exp(-x) on ScalarE, per-pair broadcast via a K=8 select-matmul
  - z pieces (256 tokens x own 512 dims) are AllGathered with the pair core
    (cheap early collectives instead of a ReduceScatter of O partials)
  - O projection contracts the full 1024 gathered z dims but only computes
    this core's 512 output columns -> rows are final, DMA straight out.
"""

import sys

import numpy as np
import ml_dtypes

if "/opt/trn_rl_repo" not in sys.path:
    sys.path.insert(0, "/opt/trn_rl_repo")

from contextlib import ExitStack

import concourse.bass as bass
import concourse.tile as tile
from concourse import mybir
import concourse.bass_utils as bass_utils

P = 128
S = 2048          # sequence length
D = 1024          # d_model
DH = 64           # head dim
NHO = 8           # heads per core
DO = 512          # own d-model slice (8 heads * 64)
NW = 1536         # own qkv output cols (512 q + 512 k + 512 v)
FCH = D // P      # 8 feature chunks (contraction over d_model)
NQC = S // 512    # 4 query chunks of 512
NPC = 8           # z-gather pieces of 256 tokens
dt = mybir.dt
AF = mybir.ActivationFunctionType


def _split_excess_waits(nc):
    """This walrus build allows 1 sync wait per instruction (2 for
    EventSemaphore); Tile's end-of-kernel drain can carry more. Move the
    extras onto preceding NoOps on the same engine."""
    for f in nc.m.functions:
        for bb in f.blocks:
            new_insts = []
            for inst in bb.instructions:
                si = inst.sync_info
                waits = list(si.on_wait) if si and si.on_wait else []
                cap = 2 if isinstance(inst, mybir.InstEventSemaphore) else 1
                if len(waits) > cap:
                    extras, keep = waits[:-cap], waits[-cap:]
                    for i, w in enumerate(extras):
                        new_insts.append(mybir.InstNoOp(
                            name=f"{inst.name}-wsplit{i}", engine=inst.engine,
                            ins=[], outs=[],
                            sync_info=mybir.SyncInfo(on_wait=[w], on_update=[])))
                    si.on_wait = keep
                new_insts.append(inst)
            bb.instructions[:] = new_insts


def _build():
    nc = bass.Bass("TRN2", target_bir_lowering=False, debug=False, num_devices=8)
    xt_d = nc.declare_dram_parameter("xt", [D, S], dt.bfloat16, isOutput=False)
    wqkv_d = nc.declare_dram_parameter("wqkv", [D, NW], dt.bfloat16, isOutput=False)
    wo_d = nc.declare_dram_parameter("wo", [D, DO], dt.bfloat16, isOutput=False)
    bqk_d = nc.declare_dram_parameter("bqk", [P, 8], dt.float32, isOutput=False)
    bv_d = nc.declare_dram_parameter("bv", [P, 4], dt.float32, isOutput=False)
    bo_d = nc.declare_dram_parameter("bo", [1, DO], dt.float32, isOutput=False)
    sels_d = nc.declare_dram_parameter("sels", [P, 1024], dt.bfloat16, isOutput=False)
    out_d = nc.declare_dram_parameter("out", [S, DO], dt.bfloat16, isOutput=True)
    zout = nc.dram_tensor("zout", [NPC, DO, 256], dt.bfloat16)
    zgath = nc.dram_tensor("zgath", [NPC, D, 256], dt.bfloat16)

    with tile.TileContext(nc) as tc, ExitStack() as ctx:
        const = ctx.enter_context(tc.tile_pool(name="const", bufs=1))
        persist = ctx.enter_context(tc.tile_pool(name="persist", bufs=1))

        # ---- constants -------------------------------------------------
        bqk_sb = const.tile([P, 8], dt.float32, name="bqk", tag="bqk")
        nc.sync.dma_start(out=bqk_sb[:], in_=bqk_d[:])
        bv_sb = const.tile([P, 4], dt.float32, name="bv", tag="bv")
        nc.sync.dma_start(out=bv_sb[:], in_=bv_d[:])
        bo_row = const.tile([1, DO], dt.float32, name="bo_row", tag="bo_row")
        nc.sync.dma_start(out=bo_row[:], in_=bo_d[:])
        bo_bc = const.tile([P, DO], dt.float32, name="bo_bc", tag="bo_bc")
        ones_col = const.tile([1, P], dt.float32, name="ones_col", tag="ones_col")
        nc.vector.memset(ones_col[:], 1.0)

        # head-pair selectors for the reciprocal broadcast matmuls: within a
        # half-batch tile, the pair's denominators sit at partitions 64*l
        # (even head) and 64*l+32 (odd head); sel block 2l/2l+1 broadcasts
        # them onto rows 0:64 / 64:128 (loaded later, off the startup path)
        sels_sb = const.tile([P, 1024], dt.bfloat16, name="sels", tag="sels")

        # triangular mask for the 128-col diagonal sub-block of each head
        # half: cm[p, j] = 1 if j >= p (two identical 128-col halves)
        ones_src = const.tile([P, 256], dt.bfloat16, name="ones_src", tag="ones_src")
        nc.gpsimd.memset(ones_src[:], 1.0)
        cmask = const.tile([P, 256], dt.bfloat16, name="cmask", tag="cmask")
        nc.gpsimd.affine_select(
            cmask[:], ones_src[:], pattern=[[0, 2], [1, P]], base=0,
            channel_multiplier=-1, compare_op=mybir.AluOpType.is_ge, fill=0.0)

        # ---- persistent activations -----------------------------------
        qT = [persist.tile([P, S], dt.bfloat16, name=f"qT{i}", tag=f"qT{i}") for i in range(4)]
        kT = [persist.tile([P, S], dt.bfloat16, name=f"kT{i}", tag=f"kT{i}") for i in range(4)]
        vv = [persist.tile([P, NHO * (DH + 1)], dt.bfloat16, name=f"vv{t}", tag=f"vv{t}")
              for t in range(S // P)]
        # all 8 heads' z in one tile so a piece export is a single 3D-AP DMA
        z_cat = persist.tile([P, 4 * S], dt.bfloat16, name="zcat", tag="zcat")
        z_all = [z_cat[:, i * S:(i + 1) * S] for i in range(4)]
        wo_bf = [persist.tile([P, DO], dt.bfloat16, name=f"wo{i}", tag=f"wo{i}")
                 for i in range(FCH)]

        # ---- pools (PSUM: proj 2 + scores 4 + zro 2 = 8 banks) --------
        ph1 = ctx.enter_context(tc.tile_pool(name="ph1", bufs=1))
        p_pool = ctx.enter_context(tc.tile_pool(name="p_pool", bufs=6))
        dn_pool = ctx.enter_context(tc.tile_pool(name="dn_pool", bufs=2))
        ost_pool = ctx.enter_context(tc.tile_pool(name="ost_pool", bufs=12))
        zg_pool = ctx.enter_context(tc.tile_pool(name="zg_pool", bufs=3))
        proj_ps = ctx.enter_context(tc.tile_pool(name="proj_ps", bufs=2, space="PSUM"))
        s_psp = ctx.enter_context(tc.tile_pool(name="s_psp", bufs=2, space="PSUM"))
        zro_psp = ctx.enter_context(tc.tile_pool(name="zro_psp", bufs=2, space="PSUM"))

        dsem = nc.alloc_semaphore("dsem")
        csem = nc.alloc_semaphore("csem")
        isem = nc.alloc_semaphore("isem")
        n_odma = [0]
        n_idma = [0]

        # broadcast b_o (own 512 cols) to all partitions via a K=1 matmul
        bps = proj_ps.tile([P, DO], dt.float32, name="bps", tag="ps")
        nc.tensor.matmul(bps[:], lhsT=ones_col[:], rhs=bo_row[:],
                         start=True, stop=True)
        nc.vector.tensor_copy(bo_bc[:], bps[:])

        xt_bf = [ph1.tile([P, S], dt.bfloat16, name=f"xt{f}", tag=f"xt{f}") for f in range(FCH)]
        wq_bf = [ph1.tile([P, NW], dt.bfloat16, name=f"wq{f}", tag=f"wq{f}") for f in range(FCH)]

        def load_w_cols(c0):
            for f in range(FCH):
                nc.sync.dma_start(
                    out=wq_bf[f][:, c0:c0 + 512],
                    in_=wqkv_d[f * P:(f + 1) * P, c0:c0 + 512])

        def load_x_cols(t):
            for f in range(FCH):
                nc.sync.dma_start(
                    out=xt_bf[f][:, t * 512:(t + 1) * 512],
                    in_=xt_d[f * P:(f + 1) * P, t * 512:(t + 1) * 512])

        def kq_proj(base, t, bias_off, dst):
            for n in range(4):
                ps = proj_ps.tile([P, 512], dt.float32, name="ps", tag="ps")
                for f in range(FCH):
                    nc.tensor.matmul(
                        ps[:], lhsT=wq_bf[f][:, base + n * P:base + (n + 1) * P],
                        rhs=xt_bf[f][:, t * 512:(t + 1) * 512],
                        start=(f == 0), stop=(f == FCH - 1))
                nc.vector.tensor_scalar_add(
                    dst[n][:, t * 512:(t + 1) * 512], ps[:],
                    bqk_sb[:, bias_off + n:bias_off + n + 1])

        def v_proj(t16):
            ps = proj_ps.tile([P, 512], dt.float32, name="ps", tag="ps")
            for f in range(FCH):
                nc.tensor.matmul(
                    ps[:], lhsT=xt_bf[f][:, t16 * P:(t16 + 1) * P],
                    rhs=wq_bf[f][:, 1024:1536],
                    start=(f == 0), stop=(f == FCH - 1))
            vview = vv[t16][:].rearrange("p (h c) -> p h c", c=DH + 1)
            nc.vector.tensor_copy(
                vview[:, :, 0:DH], ps[:].rearrange("p (h c) -> p h c", c=DH))
            nc.vector.memset(vview[:, :, DH:DH + 1], 1.0)

        def attention_pairs(qc, dn_a, dn_b, weave_o, weave_p, weave_og):
            qs = qc * 512
            n_kc = 4 * (qc + 1)
            total = 4 * n_kc
            # evenly spread this chunk's projection quota (the next token
            # block's 12 groups) and, in the last chunk, the deferred
            # O-projection groups across the kc iterations
            p_quota = min(12, len(weave_p))
            # hold back a few O-groups: they become PE filler for the tail,
            # where the PE otherwise idles on the last AllGathers
            og_quota = max(0, len(weave_og) - 6)
            p_done = og_done = 0
            inqc = 0
            for ht in range(NHO // 2):
                # heads 2*ht (rows 0:64) and 2*ht+1 (rows 64:128) share the
                # kT/qT tile; their K=64 score matmuls target disjoint PE
                # row-quadrants and run concurrently
                z0 = zro_psp.tile([DH + 1, 512], dt.float32, name="zps0", tag="zro")
                z1 = zro_psp.tile([DH + 1, 512], dt.float32, name="zps1", tag="zro")
                for kc in range(n_kc):
                    di = kc - 4 * qc   # >=0 -> diagonal block
                    off = 128 * di if di > 0 else 0
                    s_ps = s_psp.tile([P, 1024], dt.float32, name="sps", tag="sps")
                    nc.tensor.matmul(
                        s_ps[:, off:512],
                        lhsT=kT[ht][0:DH, kc * P:(kc + 1) * P],
                        rhs=qT[ht][0:DH, qs + off:qs + 512],
                        start=True, stop=True)
                    nc.tensor.matmul(
                        s_ps[:, 512 + off:1024],
                        lhsT=kT[ht][DH:P, kc * P:(kc + 1) * P],
                        rhs=qT[ht][DH:P, qs + off:qs + 512],
                        start=True, stop=True)
                    p_t = p_pool.tile([P, 1024], dt.bfloat16, name="pt", tag="pt")
                    if off:
                        sv = s_ps[:].rearrange("p (h c) -> p h c", h=2)
                        pv = p_t[:].rearrange("p (h c) -> p h c", h=2)
                        nc.scalar.activation(
                            pv[:, :, off:512], sv[:, :, off:512], AF.Exp, scale=0.125)
                    else:
                        nc.scalar.activation(p_t[:], s_ps[:], AF.Exp, scale=0.125)
                    if di >= 0:
                        # triangular mask on the single partially-valid
                        # 128-col sub-block of each head half (DVE multiply)
                        pv = p_t[:].rearrange("p (h c) -> p h c", h=2)
                        cmv = cmask[:].rearrange("p (h c) -> p h c", h=2)
                        nc.vector.tensor_tensor(
                            pv[:, :, off:off + P], pv[:, :, off:off + P],
                            cmv[:], mybir.AluOpType.mult)
                    kcnt[0] += 1
                    inqc += 1
                    if weave_o and inqc >= weave_o[0][0] and kcnt[0] % 3 == 0:
                        # previous chunk's z-import, delayed until its
                        # AllGather has landed
                        weave_o.pop(0)[1]()
                    elif weave_p and p_done < (inqc * p_quota) // total:
                        # this chunk's share of later-block projections
                        weave_p.pop(0)()
                        p_done += 1
                    elif (not weave_p and weave_og
                          and weave_og[0][0] in zg_holder
                          and og_done < (inqc * og_quota) // total):
                        # deferred O-projection groups (fill the last chunk)
                        weave_og.pop(0)[1]()
                        og_done += 1
                    nc.tensor.matmul(
                        z0[:, off:512], lhsT=vv[kc][:, (2 * ht) * 65:(2 * ht) * 65 + 65],
                        rhs=p_t[:, off:512],
                        start=(kc == 0), stop=(kc == n_kc - 1))
                    nc.tensor.matmul(
                        z1[:, off:512],
                        lhsT=vv[kc][:, (2 * ht + 1) * 65:(2 * ht + 1) * 65 + 65],
                        rhs=p_t[:, 512 + off:1024],
                        start=(kc == 0), stop=(kc == n_kc - 1))
                # copy out unnormalized z + denominator rows; normalization
                # happens in two half-batches (pairs 0-1 mid-attention,
                # pairs 2-3 at the chunk boundary)
                tgt = dn_a if ht < 2 else dn_b
                hp = 64 * (ht % 2)
                c0 = ht * S + qs
                nc.vector.tensor_copy(z_cat[0:DH, c0:c0 + 512], z0[0:DH, :])
                nc.vector.tensor_copy(
                    tgt[hp:hp + 1, :], z0[DH:DH + 1, :])
                nc.vector.tensor_copy(z_cat[DH:P, c0:c0 + 512], z1[0:DH, :])
                nc.vector.tensor_copy(
                    tgt[hp + 32:hp + 33, :], z1[DH:DH + 1, :])
                if ht == 1:
                    qc_epilogue(qc, dn_a, 0)

        def qc_epilogue(qc, dn_t, b, ps_pool=None):
            # batched softmax denominators for head-pairs (2b, 2b+1): one Ln
            # + one Exp(-x) (1/d = exp(-ln d); both live in one ACT table
            # set). ScalarE cost scales with the free dim, not partitions.
            # When woven into the next chunk, rbc must come from proj_ps (a
            # self-contained ring) -- the zro ring holds live accumulators.
            qs = qc * 512
            pool = ps_pool or zro_psp
            tg = "ps" if ps_pool else "zro"
            lnv = dn_pool.tile([P, 512], dt.float32, name="lnv", tag="lnv")
            nc.scalar.activation(lnv[:], dn_t[:], AF.Ln)
            rcp = dn_pool.tile([P, 512], dt.bfloat16, name="rcp", tag="rcp")
            nc.scalar.activation(rcp[:], lnv[:], AF.Exp, scale=-1.0)
            for ht in (2 * b, 2 * b + 1):
                l = ht % 2
                rbc = pool.tile([P, 512], dt.float32, name="rbc", tag=tg)
                nc.tensor.matmul(
                    rbc[:], lhsT=sels_sb[:, (2 * l) * P:(2 * l + 1) * P],
                    rhs=rcp[:], start=True, stop=False)
                nc.tensor.matmul(
                    rbc[:], lhsT=sels_sb[:, (2 * l + 1) * P:(2 * l + 2) * P],
                    rhs=rcp[:], start=False, stop=True)
                zsl = z_cat[:, ht * S + qs:ht * S + qs + 512]
                nc.vector.tensor_tensor(zsl, zsl, rbc[:], mybir.AluOpType.mult)
                nc.vector.tensor_scalar_add(zsl, zsl, bv_sb[:, ht:ht + 1])

        def export_ag(qc):
            # push this qc's two 256-token z pieces to DRAM (one 3D-AP DMA
            # each) and AllGather them with the pair core while later chunks
            # keep computing
            zv = z_cat[:].rearrange("p (d s) -> p d s", d=4)
            with tc.tile_critical():
                for half in range(2):
                    pi = 2 * qc + half
                    t0 = pi * 256
                    nc.gpsimd.dma_start(
                        out=zout[pi].rearrange("(d p) t -> p d t", p=P),
                        in_=zv[:, :, t0:t0 + 256]).then_inc(dsem, 16)
                    n_odma[0] += 1
                    nc.gpsimd.wait_ge(dsem, 16 * n_odma[0])
                    nc.gpsimd.collective_compute(
                        "AllGather", mybir.AluOpType.bypass,
                        replica_groups=[[0, 1], [2, 3], [4, 5], [6, 7]],
                        ins=[zout[pi]],
                        outs=[zgath[pi]],
                    ).then_inc(csem, 1)

        def import_pieces(qc, halves):
            # gathered pieces of qc, each as one 3D-AP DMA; when woven (>=18
            # kc-iters into the next chunk) the csem wait is stale, so the
            # critical barrier is brief
            zg_t = {}
            for half in halves:
                zg_t[half] = zg_pool.tile(
                    [P, 2048], dt.bfloat16, name=f"zg{half}", tag=f"zg{half}")
            with tc.tile_critical():
                nc.sync.wait_ge(csem, 2 * qc + 1 + max(halves))
                for half in halves:
                    pi = 2 * qc + half
                    nc.sync.dma_start(
                        out=zg_t[half][:].rearrange("p (d t) -> p d t", d=FCH),
                        in_=zgath[pi].rearrange("(d p) t -> p d t", p=P),
                    ).then_inc(isem, 16)
                    n_idma[0] += 1
                nc.sync.wait_ge(isem, 16 * n_idma[0])
            return zg_t

        def o_group(pi, g, zg):
            tok = pi * 256 + g * P
            ps = zro_psp.tile([P, DO], dt.float32, name="ops", tag="zro")
            for d in range(FCH):
                nc.tensor.matmul(
                    ps[:], lhsT=zg[:, d * 256 + g * P:d * 256 + (g + 1) * P],
                    rhs=wo_bf[d][:], start=(d == 0), stop=(d == FCH - 1))
            ost = ost_pool.tile([P, DO], dt.bfloat16, name="ost", tag="ost")
            nc.vector.tensor_tensor(ost[:], ps[:], bo_bc[:], mybir.AluOpType.add)
            nc.sync.dma_start(out=out_d[tok:tok + P, :], in_=ost[:])

        def o_thunks(qc):
            # import thunk (woven into the next chunk once the AllGather
            # has landed) + O-projection groups (deferred to fill the last
            # chunk's otherwise idle PE)
            def imp(qc=qc):
                zg_holder[qc] = import_pieces(qc, (0, 1))
            og = []
            for half in range(2):
                for g in range(2):
                    og.append((qc, lambda qc=qc, half=half, g=g: o_group(
                        2 * qc + half, g, zg_holder[qc][half])))
            return [(18, imp)], og

        def o_direct(qc):
            # tail: per-piece import so piece 0's O-projection overlaps
            # piece 1's AllGather
            for half in range(2):
                zg_t = import_pieces(qc, (half,))
                for g in range(2):
                    o_group(2 * qc + half, g, zg_t[half])

        # Emission strategy: the attention phase is ScalarE-bound (one exp
        # per kc tile paces it) while the projections are TensorE-bound. All
        # engines execute their streams in order, so to overlap the phases
        # the projection groups of later token blocks are woven INTO the
        # attention kc-loops as PE filler work.
        # K-weight and x(t0) chunks land pairwise so the first projection
        # matmul starts after two DMAs instead of sixteen.
        for f in range(FCH):
            nc.sync.dma_start(
                out=wq_bf[f][:, 512:1024],
                in_=wqkv_d[f * P:(f + 1) * P, 512:1024])
            nc.sync.dma_start(
                out=xt_bf[f][:, 0:512], in_=xt_d[f * P:(f + 1) * P, 0:512])
        load_w_cols(0)                # Q weight columns
        load_w_cols(1024)             # V weight columns
        for t in range(1, 4):
            load_x_cols(t)
        nc.sync.dma_start(out=sels_sb[:], in_=sels_d[:])
        for dc in range(FCH):
            nc.sync.dma_start(out=wo_bf[dc][:], in_=wo_d[dc * P:(dc + 1) * P, :])

        # prime the combined ln+exp ACT table set before the exp stream so
        # the per-chunk Ln does not force a mid-kernel table switch
        lnjunk = const.tile([1, 8], dt.float32, name="lnjunk", tag="lnjunk")
        nc.scalar.activation(lnjunk[:], bo_bc[0:1, 0:8], AF.Ln)
        nc.scalar.activation(lnjunk[:], lnjunk[:], AF.Exp)

        def kq_one(base, n, t, bias_off, dst):
            def f():
                ps = proj_ps.tile([P, 512], dt.float32, name="ps", tag="ps")
                for fc in range(FCH):
                    nc.tensor.matmul(
                        ps[:], lhsT=wq_bf[fc][:, base + n * P:base + (n + 1) * P],
                        rhs=xt_bf[fc][:, t * 512:(t + 1) * 512],
                        start=(fc == 0), stop=(fc == FCH - 1))
                nc.vector.tensor_scalar_add(
                    dst[n][:, t * 512:(t + 1) * 512], ps[:],
                    bqk_sb[:, bias_off + n:bias_off + n + 1])
            return f

        # token block 0 projected up front (nothing to overlap with yet)
        kq_proj(512, 0, 4, kT)
        kq_proj(0, 0, 0, qT)
        for t16 in range(4):
            v_proj(t16)

        # later blocks become weave thunks, ordered by token block
        kcnt = [0]
        zg_holder = {}
        weave_o = []
        weave_og = []
        weave_p = []
        v3_thunks = []
        for t in range(1, 4):
            for n in range(4):
                weave_p.append(kq_one(512, n, t, 4, kT))
                weave_p.append(kq_one(0, n, t, 0, qT))
            for t16 in range(4 * t, 4 * t + 4):
                th = (lambda tt: lambda: v_proj(tt))(t16)
                if t == 3:
                    # V(t=3) is only read from kc block 12 on -- weave it
                    # into the last chunk's early iterations as PE filler
                    v3_thunks.append((2 * (t16 - 11), th))
                else:
                    weave_p.append(th)
        n_wp = len(weave_p)
        need_through = [0, 12, 24, n_wp]

        for qc in range(NQC):
            # projection groups the sparse weave has not placed yet must be
            # emitted before the attention that reads them (block t=qc)
            while weave_p and n_wp - len(weave_p) < need_through[qc]:
                weave_p.pop(0)()
            dn_a = dn_pool.tile([P, 512], dt.float32, name="dna", tag="dna")
            dn_b = dn_pool.tile([P, 512], dt.float32, name="dnb", tag="dnb")
            nc.vector.memset(dn_a[:], 1.0)
            nc.vector.memset(dn_b[:], 1.0)
            attention_pairs(qc, dn_a, dn_b, weave_o, weave_p, weave_og)
            while weave_o:
                # leftover thunks of the previous chunk
                weave_o.pop(0)[1]()
            if qc < NQC - 1:
                # normalize heads 2-3 and export+AllGather a few iterations
                # INTO the next chunk, so its attention starts immediately
                weave_o, og = o_thunks(qc)
                weave_o = [(1, lambda qc=qc, dn_b=dn_b: qc_epilogue(
                               qc, dn_b, 1, ps_pool=proj_ps)),
                           (5, lambda qc=qc: export_ag(qc))] + weave_o
                if qc == NQC - 2:
                    weave_o = sorted(v3_thunks + weave_o,
                                     key=lambda t: t[0])
                weave_og.extend(og)
            else:
                qc_epilogue(qc, dn_b, 1)
                export_ag(qc)
                while weave_og:
                    weave_og.pop(0)[1]()
                o_direct(qc)
        while weave_p:
            weave_p.pop(0)()

    _split_excess_waits(nc)
    return nc


_NC = {}


def _get_nc():
    if "nc" not in _NC:
        _NC["nc"] = _build()
    return _NC["nc"]


def _shard(inputs):
    x = np.ascontiguousarray(inputs["x"], dtype=np.float32)
    W_qkv = np.asarray(inputs["W_qkv"], dtype=np.float32)
    b_qkv = np.asarray(inputs["b_qkv"], dtype=np.float32)
    W_o = np.asarray(inputs["W_o"], dtype=np.float32)
    b_o = np.asarray(inputs["b_o"], dtype=np.float32)

    in_maps = []
    for c in range(8):
        b, hh = c // 2, c % 2
        sl = slice(hh * DO, (hh + 1) * DO)
        wq = W_qkv[sl]
        wk = W_qkv[D + hh * DO:D + hh * DO + DO]
        wv = W_qkv[2 * D + hh * DO:2 * D + hh * DO + DO]
        wqkvT = np.ascontiguousarray(np.concatenate([wq, wk, wv], axis=0).T)
        bqk = np.ascontiguousarray(
            np.concatenate([b_qkv[hh * DO:hh * DO + DO],
                            b_qkv[D + hh * DO:D + hh * DO + DO]])
            .reshape(8, P).T)
        bv = np.ascontiguousarray(
            b_qkv[2 * D + hh * DO:2 * D + hh * DO + DO].reshape(4, P).T)
        # O projection: full 1024 contraction rows, own 512 output columns
        woT = np.ascontiguousarray(W_o.T[:, sl])
        sels = np.zeros((P, 1024), dtype=np.float32)
        for l in range(2):
            sels[64 * l, (2 * l) * P:(2 * l) * P + DH] = 1.0
            sels[64 * l + 32, (2 * l + 1) * P + DH:(2 * l + 2) * P] = 1.0
        in_maps.append({
            "xt": np.ascontiguousarray(x[b].T).astype(ml_dtypes.bfloat16),
            "wqkv": wqkvT.astype(ml_dtypes.bfloat16),
            "wo": woT.astype(ml_dtypes.bfloat16),
            "bqk": bqk,
            "bv": bv,
            "bo": np.ascontiguousarray(b_o[sl].reshape(1, DO)),
            "sels": sels.astype(ml_dtypes.bfloat16),
        })
    return in_maps


def _unshard(results, batch):
    out = np.empty((batch, S, D), dtype=np.float32)
    for b in range(batch):
        out[b, :, 0:DO] = results[2 * b]["out"].astype(np.float32)
        out[b, :, DO:D] = results[2 * b + 1]["out"].astype(np.float32)
    return out


def _run(inputs, trace=False, trace_kwargs=None):
    nc = _get_nc()
    in_maps = _shard(inputs)
    if trace:
        import types
        if "antenv.axon_hooks" not in sys.modules:
            mod = types.ModuleType("antenv.axon_hooks")
            _hook = [None]
            mod.set_axon_ntff_profile_hook = lambda h: _hook.__setitem__(0, h)
            mod.get_axon_ntff_profile_hook = lambda: _hook[0]
            sys.modules["antenv.axon_hooks"] = mod
            from trn_agent_boot.trn_boot import _ntff_profile_via_ctypes
            mod.set_axon_ntff_profile_hook(
                _ntff_profile_via_ctypes("/opt/axon/libaxon_pjrt.so"))
        bass_utils.upload_artifacts = lambda tmpdir: tmpdir
    res = bass_utils.run_bass_kernel_spmd(
        nc, in_maps, core_ids=list(range(8)), trace=trace,
        **(trace_kwargs or {}))
    out = _unshard(res.results, inputs["x"].shape[0])
    return out, res


def kernel(**inputs) -> np.ndarray:
    out, _ = _run(inputs, trace=False)
    return out


# revision 56
# speedup vs baseline: 1.2110x; 1.2110x over previous
"""Causal multi-head attention block (b=4, s=2048, d=1024, 16 heads) on 8
Trainium2 NeuronCores.

Sharding: tensor-parallel over heads x data-parallel over batch.
Core c handles batch c//2 and head-half c%2 (8 of 16 heads):
  - QKV projection for its 8 heads over all 2048 tokens (bf16 matmuls,
    fp32 PSUM accumulation)
  - causal attention in [k, q] score layout: scores for the even/odd head of
    a pair run concurrently in disjoint PE row-quadrants; softmax denominator
    comes for free from a ones-column appended to the V stationary; diagonal
    key-blocks restrict score/exp/PV work to the causally-valid query columns
    and multiply a single [128, 2x128] triangular mask on P' (DVE)
  - per-qc batched normalization: denominator rows staged to [8, 512], one
    Ln + one Exp(-x) on ScalarE, per-pair broadcast via a K=8 select-matmul
  - z pieces (256 tokens x own 512 dims) are AllGathered with the pair core
    (cheap early collectives instead of a ReduceScatter of O partials)
  - O projection contracts the full 1024 gathered z dims but only computes
    this core's 512 output columns -> rows are final, DMA straight out.
"""

import sys

import numpy as np
import ml_dtypes

if "/opt/trn_rl_repo" not in sys.path:
    sys.path.insert(0, "/opt/trn_rl_repo")

from contextlib import ExitStack

import concourse.bass as bass
import concourse.tile as tile
from concourse import mybir
import concourse.bass_utils as bass_utils

P = 128
S = 2048          # sequence length
D = 1024          # d_model
DH = 64           # head dim
NHO = 8           # heads per core
DO = 512          # own d-model slice (8 heads * 64)
NW = 1536         # own qkv output cols (512 q + 512 k + 512 v)
FCH = D // P      # 8 feature chunks (contraction over d_model)
NQC = S // 512    # 4 query chunks of 512
NPC = 8           # z-gather pieces of 256 tokens
dt = mybir.dt
AF = mybir.ActivationFunctionType


def _split_excess_waits(nc):
    """This walrus build allows 1 sync wait per instruction (2 for
    EventSemaphore); Tile's end-of-kernel drain can carry more. Move the
    extras onto preceding NoOps on the same engine."""
    for f in nc.m.functions:
        for bb in f.blocks:
            new_insts = []
            for inst in bb.instructions:
                si = inst.sync_info
                waits = list(si.on_wait) if si and si.on_wait else []
                cap = 2 if isinstance(inst, mybir.InstEventSemaphore) else 1
                if len(waits) > cap:
                    extras, keep = waits[:-cap], waits[-cap:]
                    for i, w in enumerate(extras):
                        new_insts.append(mybir.InstNoOp(
                            name=f"{inst.name}-wsplit{i}", engine=inst.engine,
                            ins=[], outs=[],
                            sync_info=mybir.SyncInfo(on_wait=[w], on_update=[])))
                    si.on_wait = keep
                new_insts.append(inst)
            bb.instructions[:] = new_insts


def _build():
    nc = bass.Bass("TRN2", target_bir_lowering=False, debug=False, num_devices=8)
    xt_d = nc.declare_dram_parameter("xt", [D, S], dt.bfloat16, isOutput=False)
    wqkv_d = nc.declare_dram_parameter("wqkv", [D, NW], dt.bfloat16, isOutput=False)
    wo_d = nc.declare_dram_parameter("wo", [D, DO], dt.bfloat16, isOutput=False)
    bqk_d = nc.declare_dram_parameter("bqk", [P, 8], dt.float32, isOutput=False)
    bv_d = nc.declare_dram_parameter("bv", [P, 4], dt.float32, isOutput=False)
    bo_d = nc.declare_dram_parameter("bo", [1, DO], dt.float32, isOutput=False)
    sels_d = nc.declare_dram_parameter("sels", [P, 1024], dt.bfloat16, isOutput=False)
    out_d = nc.declare_dram_parameter("out", [S, DO], dt.bfloat16, isOutput=True)
    zout = nc.dram_tensor("zout", [NPC, DO, 256], dt.bfloat16)
    zgath = nc.dram_tensor("zgath", [NPC, D, 256], dt.bfloat16)

    with tile.TileContext(nc) as tc, ExitStack() as ctx:
        const = ctx.enter_context(tc.tile_pool(name="const", bufs=1))
        persist = ctx.enter_context(tc.tile_pool(name="persist", bufs=1))

        # ---- constants -------------------------------------------------
        bqk_sb = const.tile([P, 8], dt.float32, name="bqk", tag="bqk")
        nc.sync.dma_start(out=bqk_sb[:], in_=bqk_d[:])
        bv_sb = const.tile([P, 4], dt.float32, name="bv", tag="bv")
        nc.sync.dma_start(out=bv_sb[:], in_=bv_d[:])
        bo_row = const.tile([1, DO], dt.float32, name="bo_row", tag="bo_row")
        nc.sync.dma_start(out=bo_row[:], in_=bo_d[:])
        bo_bc = const.tile([P, DO], dt.float32, name="bo_bc", tag="bo_bc")
        ones_col = const.tile([1, P], dt.float32, name="ones_col", tag="ones_col")
        nc.vector.memset(ones_col[:], 1.0)

        # head-pair selectors for the reciprocal broadcast matmuls: within a
        # half-batch tile, the pair's denominators sit at partitions 64*l
        # (even head) and 64*l+32 (odd head); sel block 2l/2l+1 broadcasts
        # them onto rows 0:64 / 64:128 (loaded later, off the startup path)
        sels_sb = const.tile([P, 1024], dt.bfloat16, name="sels", tag="sels")

        # triangular mask for the 128-col diagonal sub-block of each head
        # half: cm[p, j] = 1 if j >= p (two identical 128-col halves)
        ones_src = const.tile([P, 256], dt.bfloat16, name="ones_src", tag="ones_src")
        nc.gpsimd.memset(ones_src[:], 1.0)
        cmask = const.tile([P, 256], dt.bfloat16, name="cmask", tag="cmask")
        nc.gpsimd.affine_select(
            cmask[:], ones_src[:], pattern=[[0, 2], [1, P]], base=0,
            channel_multiplier=-1, compare_op=mybir.AluOpType.is_ge, fill=0.0)

        # ---- persistent activations -----------------------------------
        qT = [persist.tile([P, S], dt.bfloat16, name=f"qT{i}", tag=f"qT{i}") for i in range(4)]
        kT = [persist.tile([P, S], dt.bfloat16, name=f"kT{i}", tag=f"kT{i}") for i in range(4)]
        vv = [persist.tile([P, NHO * (DH + 1)], dt.bfloat16, name=f"vv{t}", tag=f"vv{t}")
              for t in range(S // P)]
        # all 8 heads' z in one tile so a piece export is a single 3D-AP DMA
        z_cat = persist.tile([P, 4 * S], dt.bfloat16, name="zcat", tag="zcat")
        z_all = [z_cat[:, i * S:(i + 1) * S] for i in range(4)]
        wo_bf = [persist.tile([P, DO], dt.bfloat16, name=f"wo{i}", tag=f"wo{i}")
                 for i in range(FCH)]

        # ---- pools (PSUM: proj 2 + scores 4 + zro 2 = 8 banks) --------
        ph1 = ctx.enter_context(tc.tile_pool(name="ph1", bufs=1))
        p_pool = ctx.enter_context(tc.tile_pool(name="p_pool", bufs=6))
        dn_pool = ctx.enter_context(tc.tile_pool(name="dn_pool", bufs=2))
        ost_pool = ctx.enter_context(tc.tile_pool(name="ost_pool", bufs=12))
        zg_pool = ctx.enter_context(tc.tile_pool(name="zg_pool", bufs=3))
        proj_ps = ctx.enter_context(tc.tile_pool(name="proj_ps", bufs=2, space="PSUM"))
        s_psp = ctx.enter_context(tc.tile_pool(name="s_psp", bufs=2, space="PSUM"))
        zro_psp = ctx.enter_context(tc.tile_pool(name="zro_psp", bufs=2, space="PSUM"))

        dsem = nc.alloc_semaphore("dsem")
        csem = nc.alloc_semaphore("csem")
        isem = nc.alloc_semaphore("isem")
        n_odma = [0]
        n_idma = [0]

        # broadcast b_o (own 512 cols) to all partitions via a K=1 matmul
        bps = proj_ps.tile([P, DO], dt.float32, name="bps", tag="ps")
        nc.tensor.matmul(bps[:], lhsT=ones_col[:], rhs=bo_row[:],
                         start=True, stop=True)
        nc.vector.tensor_copy(bo_bc[:], bps[:])

        xt_bf = [ph1.tile([P, S], dt.bfloat16, name=f"xt{f}", tag=f"xt{f}") for f in range(FCH)]
        wq_bf = [ph1.tile([P, NW], dt.bfloat16, name=f"wq{f}", tag=f"wq{f}") for f in range(FCH)]

        def load_w_cols(c0):
            for f in range(FCH):
                nc.sync.dma_start(
                    out=wq_bf[f][:, c0:c0 + 512],
                    in_=wqkv_d[f * P:(f + 1) * P, c0:c0 + 512])

        def load_x_cols(t):
            for f in range(FCH):
                nc.sync.dma_start(
                    out=xt_bf[f][:, t * 512:(t + 1) * 512],
                    in_=xt_d[f * P:(f + 1) * P, t * 512:(t + 1) * 512])

        def kq_proj(base, t, bias_off, dst):
            for n in range(4):
                ps = proj_ps.tile([P, 512], dt.float32, name="ps", tag="ps")
                for f in range(FCH):
                    nc.tensor.matmul(
                        ps[:], lhsT=wq_bf[f][:, base + n * P:base + (n + 1) * P],
                        rhs=xt_bf[f][:, t * 512:(t + 1) * 512],
                        start=(f == 0), stop=(f == FCH - 1))
                nc.vector.tensor_scalar_add(
                    dst[n][:, t * 512:(t + 1) * 512], ps[:],
                    bqk_sb[:, bias_off + n:bias_off + n + 1])

        def v_proj(t16):
            ps = proj_ps.tile([P, 512], dt.float32, name="ps", tag="ps")
            for f in range(FCH):
                nc.tensor.matmul(
                    ps[:], lhsT=xt_bf[f][:, t16 * P:(t16 + 1) * P],
                    rhs=wq_bf[f][:, 1024:1536],
                    start=(f == 0), stop=(f == FCH - 1))
            vview = vv[t16][:].rearrange("p (h c) -> p h c", c=DH + 1)
            nc.vector.tensor_copy(
                vview[:, :, 0:DH], ps[:].rearrange("p (h c) -> p h c", c=DH))
            nc.vector.memset(vview[:, :, DH:DH + 1], 1.0)

        def attention_pairs(qc, dn_a, dn_b, weave_o, weave_p, weave_og):
            qs = qc * 512
            n_kc = 4 * (qc + 1)
            total = 4 * n_kc
            # evenly spread this chunk's projection quota (the next token
            # block's 12 groups) and, in the last chunk, the deferred
            # O-projection groups across the kc iterations
            p_quota = min(12, len(weave_p))
            # hold back a few O-groups: they become PE filler for the tail,
            # where the PE otherwise idles on the last AllGathers
            og_quota = max(0, len(weave_og) - 6)
            p_done = og_done = 0
            inqc = 0
            for ht in range(NHO // 2):
                # heads 2*ht (rows 0:64) and 2*ht+1 (rows 64:128) share the
                # kT/qT tile; their K=64 score matmuls target disjoint PE
                # row-quadrants and run concurrently
                z0 = zro_psp.tile([DH + 1, 512], dt.float32, name="zps0", tag="zro")
                z1 = zro_psp.tile([DH + 1, 512], dt.float32, name="zps1", tag="zro")
                for kc in range(n_kc):
                    di = kc - 4 * qc   # >=0 -> diagonal block
                    off = 128 * di if di > 0 else 0
                    s_ps = s_psp.tile([P, 1024], dt.float32, name="sps", tag="sps")
                    nc.tensor.matmul(
                        s_ps[:, off:512],
                        lhsT=kT[ht][0:DH, kc * P:(kc + 1) * P],
                        rhs=qT[ht][0:DH, qs + off:qs + 512],
                        start=True, stop=True)
                    nc.tensor.matmul(
                        s_ps[:, 512 + off:1024],
                        lhsT=kT[ht][DH:P, kc * P:(kc + 1) * P],
                        rhs=qT[ht][DH:P, qs + off:qs + 512],
                        start=True, stop=True)
                    p_t = p_pool.tile([P, 1024], dt.bfloat16, name="pt", tag="pt")
                    if off:
                        sv = s_ps[:].rearrange("p (h c) -> p h c", h=2)
                        pv = p_t[:].rearrange("p (h c) -> p h c", h=2)
                        nc.scalar.activation(
                            pv[:, :, off:512], sv[:, :, off:512], AF.Exp, scale=0.125)
                    else:
                        nc.scalar.activation(p_t[:], s_ps[:], AF.Exp, scale=0.125)
                    if di >= 0:
                        # triangular mask on the single partially-valid
                        # 128-col sub-block of each head half (DVE multiply)
                        pv = p_t[:].rearrange("p (h c) -> p h c", h=2)
                        cmv = cmask[:].rearrange("p (h c) -> p h c", h=2)
                        nc.vector.tensor_tensor(
                            pv[:, :, off:off + P], pv[:, :, off:off + P],
                            cmv[:], mybir.AluOpType.mult)
                    kcnt[0] += 1
                    inqc += 1
                    if weave_o and inqc >= weave_o[0][0] and kcnt[0] % 3 == 0:
                        # previous chunk's z-import, delayed until its
                        # AllGather has landed
                        weave_o.pop(0)[1]()
                    elif weave_p and p_done < (inqc * p_quota) // total:
                        # this chunk's share of later-block projections
                        weave_p.pop(0)()
                        p_done += 1
                    elif (not weave_p and weave_og
                          and weave_og[0][0] in zg_holder
                          and og_done < (inqc * og_quota) // total):
                        # deferred O-projection groups (fill the last chunk)
                        weave_og.pop(0)[1]()
                        og_done += 1
                    nc.tensor.matmul(
                        z0[:, off:512], lhsT=vv[kc][:, (2 * ht) * 65:(2 * ht) * 65 + 65],
                        rhs=p_t[:, off:512],
                        start=(kc == 0), stop=(kc == n_kc - 1))
                    nc.tensor.matmul(
                        z1[:, off:512],
                        lhsT=vv[kc][:, (2 * ht + 1) * 65:(2 * ht + 1) * 65 + 65],
                        rhs=p_t[:, 512 + off:1024],
                        start=(kc == 0), stop=(kc == n_kc - 1))
                # copy out unnormalized z + denominator rows; normalization
                # happens in two half-batches (pairs 0-1 mid-attention,
                # pairs 2-3 at the chunk boundary)
                tgt = dn_a if ht < 2 else dn_b
                hp = 64 * (ht % 2)
                c0 = ht * S + qs
                nc.vector.tensor_copy(z_cat[0:DH, c0:c0 + 512], z0[0:DH, :])
                nc.vector.tensor_copy(
                    tgt[hp:hp + 1, :], z0[DH:DH + 1, :])
                nc.vector.tensor_copy(z_cat[DH:P, c0:c0 + 512], z1[0:DH, :])
                nc.vector.tensor_copy(
                    tgt[hp + 32:hp + 33, :], z1[DH:DH + 1, :])
                if ht == 1:
                    qc_epilogue(qc, dn_a, 0)

        def qc_epilogue(qc, dn_t, b, ps_pool=None):
            # batched softmax denominators for head-pairs (2b, 2b+1): one Ln
            # + one Exp(-x) (1/d = exp(-ln d); both live in one ACT table
            # set). ScalarE cost scales with the free dim, not partitions.
            # When woven into the next chunk, rbc must come from proj_ps (a
            # self-contained ring) -- the zro ring holds live accumulators.
            qs = qc * 512
            pool = ps_pool or zro_psp
            tg = "ps" if ps_pool else "zro"
            lnv = dn_pool.tile([P, 512], dt.float32, name="lnv", tag="lnv")
            nc.scalar.activation(lnv[:], dn_t[:], AF.Ln)
            rcp = dn_pool.tile([P, 512], dt.bfloat16, name="rcp", tag="rcp")
            nc.scalar.activation(rcp[:], lnv[:], AF.Exp, scale=-1.0)
            for ht in (2 * b, 2 * b + 1):
                l = ht % 2
                rbc = pool.tile([P, 512], dt.float32, name="rbc", tag=tg)
                nc.tensor.matmul(
                    rbc[:], lhsT=sels_sb[:, (2 * l) * P:(2 * l + 1) * P],
                    rhs=rcp[:], start=True, stop=False)
                nc.tensor.matmul(
                    rbc[:], lhsT=sels_sb[:, (2 * l + 1) * P:(2 * l + 2) * P],
                    rhs=rcp[:], start=False, stop=True)
                zsl = z_cat[:, ht * S + qs:ht * S + qs + 512]
                nc.vector.tensor_tensor(zsl, zsl, rbc[:], mybir.AluOpType.mult)
                nc.vector.tensor_scalar_add(zsl, zsl, bv_sb[:, ht:ht + 1])

        def export_ag(qc):
            # push this qc's two 256-token z pieces to DRAM (one 3D-AP DMA
            # each) and AllGather them with the pair core while later chunks
            # keep computing
            zv = z_cat[:].rearrange("p (d s) -> p d s", d=4)
            with tc.tile_critical():
                for half in range(2):
                    pi = 2 * qc + half
                    t0 = pi * 256
                    nc.gpsimd.dma_start(
                        out=zout[pi].rearrange("(d p) t -> p d t", p=P),
                        in_=zv[:, :, t0:t0 + 256]).then_inc(dsem, 16)
                    n_odma[0] += 1
                    nc.gpsimd.wait_ge(dsem, 16 * n_odma[0])
                    nc.gpsimd.collective_compute(
                        "AllGather", mybir.AluOpType.bypass,
                        replica_groups=[[0, 1], [2, 3], [4, 5], [6, 7]],
                        ins=[zout[pi]],
                        outs=[zgath[pi]],
                    ).then_inc(csem, 1)

        def import_pieces(qc, halves):
            # gathered pieces of qc, each as one 3D-AP DMA; when woven (>=18
            # kc-iters into the next chunk) the csem wait is stale, so the
            # critical barrier is brief
            zg_t = {}
            for half in halves:
                zg_t[half] = zg_pool.tile(
                    [P, 2048], dt.bfloat16, name=f"zg{half}", tag=f"zg{half}")
            with tc.tile_critical():
                nc.sync.wait_ge(csem, 2 * qc + 1 + max(halves))
                for half in halves:
                    pi = 2 * qc + half
                    nc.sync.dma_start(
                        out=zg_t[half][:].rearrange("p (d t) -> p d t", d=FCH),
                        in_=zgath[pi].rearrange("(d p) t -> p d t", p=P),
                    ).then_inc(isem, 16)
                    n_idma[0] += 1
                nc.sync.wait_ge(isem, 16 * n_idma[0])
            return zg_t

        def o_group(pi, g, zg):
            tok = pi * 256 + g * P
            ps = zro_psp.tile([P, DO], dt.float32, name="ops", tag="zro")
            for d in range(FCH):
                nc.tensor.matmul(
                    ps[:], lhsT=zg[:, d * 256 + g * P:d * 256 + (g + 1) * P],
                    rhs=wo_bf[d][:], start=(d == 0), stop=(d == FCH - 1))
            ost = ost_pool.tile([P, DO], dt.bfloat16, name="ost", tag="ost")
            nc.vector.tensor_tensor(ost[:], ps[:], bo_bc[:], mybir.AluOpType.add)
            nc.sync.dma_start(out=out_d[tok:tok + P, :], in_=ost[:])

        def o_thunks(qc):
            # import thunk (woven into the next chunk once the AllGather
            # has landed) + O-projection groups (deferred to fill the last
            # chunk's otherwise idle PE)
            def imp(qc=qc):
                zg_holder[qc] = import_pieces(qc, (0, 1))
            og = []
            for half in range(2):
                for g in range(2):
                    og.append((qc, lambda qc=qc, half=half, g=g: o_group(
                        2 * qc + half, g, zg_holder[qc][half])))
            return [(18, imp)], og

        def o_direct(qc):
            # tail: per-piece import so piece 0's O-projection overlaps
            # piece 1's AllGather
            for half in range(2):
                zg_t = import_pieces(qc, (half,))
                for g in range(2):
                    o_group(2 * qc + half, g, zg_t[half])

        # Emission strategy: the attention phase is ScalarE-bound (one exp
        # per kc tile paces it) while the projections are TensorE-bound. All
        # engines execute their streams in order, so to overlap the phases
        # the projection groups of later token blocks are woven INTO the
        # attention kc-loops as PE filler work.
        # K-weight and x(t0) chunks land pairwise so the first projection
        # matmul starts after two DMAs instead of sixteen.
        for f in range(FCH):
            nc.sync.dma_start(
                out=wq_bf[f][:, 512:1024],
                in_=wqkv_d[f * P:(f + 1) * P, 512:1024])
            nc.sync.dma_start(
                out=xt_bf[f][:, 0:512], in_=xt_d[f * P:(f + 1) * P, 0:512])
        load_w_cols(0)                # Q weight columns
        load_w_cols(1024)             # V weight columns
        for t in range(1, 4):
            load_x_cols(t)
        nc.sync.dma_start(out=sels_sb[:], in_=sels_d[:])
        for dc in range(FCH):
            nc.sync.dma_start(out=wo_bf[dc][:], in_=wo_d[dc * P:(dc + 1) * P, :])

        # prime the combined ln+exp ACT table set before the exp stream so
        # the per-chunk Ln does not force a mid-kernel table switch
        lnjunk = const.tile([1, 8], dt.float32, name="lnjunk", tag="lnjunk")
        nc.scalar.activation(lnjunk[:], bo_bc[0:1, 0:8], AF.Ln)
        nc.scalar.activation(lnjunk[:], lnjunk[:], AF.Exp)

        def kq_one(base, n, t, bias_off, dst):
            def f():
                ps = proj_ps.tile([P, 512], dt.float32, name="ps", tag="ps")
                for fc in range(FCH):
                    nc.tensor.matmul(
                        ps[:], lhsT=wq_bf[fc][:, base + n * P:base + (n + 1) * P],
                        rhs=xt_bf[fc][:, t * 512:(t + 1) * 512],
                        start=(fc == 0), stop=(fc == FCH - 1))
                nc.vector.tensor_scalar_add(
                    dst[n][:, t * 512:(t + 1) * 512], ps[:],
                    bqk_sb[:, bias_off + n:bias_off + n + 1])
            return f

        # token block 0 projected up front (nothing to overlap with yet)
        kq_proj(512, 0, 4, kT)
        kq_proj(0, 0, 0, qT)
        for t16 in range(4):
            v_proj(t16)

        # later blocks become weave thunks, ordered by token block
        kcnt = [0]
        zg_holder = {}
        weave_o = []
        weave_og = []
        weave_p = []
        v3_thunks = []
        for t in range(1, 4):
            for n in range(4):
                weave_p.append(kq_one(512, n, t, 4, kT))
                weave_p.append(kq_one(0, n, t, 0, qT))
            for t16 in range(4 * t, 4 * t + 4):
                th = (lambda tt: lambda: v_proj(tt))(t16)
                if t == 3:
                    # V(t=3) is only read from kc block 12 on -- weave it
                    # into the last chunk's early iterations as PE filler
                    v3_thunks.append((2 * (t16 - 11), th))
                else:
                    weave_p.append(th)
        n_wp = len(weave_p)
        need_through = [0, 12, 24, n_wp]

        for qc in range(NQC):
            # projection groups the sparse weave has not placed yet must be
            # emitted before the attention that reads them (block t=qc)
            while weave_p and n_wp - len(weave_p) < need_through[qc]:
                weave_p.pop(0)()
            dn_a = dn_pool.tile([P, 512], dt.float32, name="dna", tag="dna")
            dn_b = dn_pool.tile([P, 512], dt.float32, name="dnb", tag="dnb")
            nc.vector.memset(dn_a[:], 1.0)
            nc.vector.memset(dn_b[:], 1.0)
            attention_pairs(qc, dn_a, dn_b, weave_o, weave_p, weave_og)
            while weave_o:
                # leftover thunks of the previous chunk
                weave_o.pop(0)[1]()
            qc_epilogue(qc, dn_b, 1)
            export_ag(qc)
            if qc < NQC - 1:
                weave_o, og = o_thunks(qc)
                if qc == NQC - 2:
                    weave_o = sorted(v3_thunks + weave_o,
                                     key=lambda t: t[0])
                weave_og.extend(og)
            else:
                while weave_og:
                    weave_og.pop(0)[1]()
                o_direct(qc)
        while weave_p:
            weave_p.pop(0)()

    _split_excess_waits(nc)
    return nc


_NC = {}


def _get_nc():
    if "nc" not in _NC:
        _NC["nc"] = _build()
    return _NC["nc"]


def _shard(inputs):
    x = np.ascontiguousarray(inputs["x"], dtype=np.float32)
    W_qkv = np.asarray(inputs["W_qkv"], dtype=np.float32)
    b_qkv = np.asarray(inputs["b_qkv"], dtype=np.float32)
    W_o = np.asarray(inputs["W_o"], dtype=np.float32)
    b_o = np.asarray(inputs["b_o"], dtype=np.float32)

    in_maps = []
    for c in range(8):
        b, hh = c // 2, c % 2
        sl = slice(hh * DO, (hh + 1) * DO)
        wq = W_qkv[sl]
        wk = W_qkv[D + hh * DO:D + hh * DO + DO]
        wv = W_qkv[2 * D + hh * DO:2 * D + hh * DO + DO]
        wqkvT = np.ascontiguousarray(np.concatenate([wq, wk, wv], axis=0).T)
        bqk = np.ascontiguousarray(
            np.concatenate([b_qkv[hh * DO:hh * DO + DO],
                            b_qkv[D + hh * DO:D + hh * DO + DO]])
            .reshape(8, P).T)
        bv = np.ascontiguousarray(
            b_qkv[2 * D + hh * DO:2 * D + hh * DO + DO].reshape(4, P).T)
        # O projection: full 1024 contraction rows, own 512 output columns
        woT = np.ascontiguousarray(W_o.T[:, sl])
        sels = np.zeros((P, 1024), dtype=np.float32)
        for l in range(2):
            sels[64 * l, (2 * l) * P:(2 * l) * P + DH] = 1.0
            sels[64 * l + 32, (2 * l + 1) * P + DH:(2 * l + 2) * P] = 1.0
        in_maps.append({
            "xt": np.ascontiguousarray(x[b].T).astype(ml_dtypes.bfloat16),
            "wqkv": wqkvT.astype(ml_dtypes.bfloat16),
            "wo": woT.astype(ml_dtypes.bfloat16),
            "bqk": bqk,
            "bv": bv,
            "bo": np.ascontiguousarray(b_o[sl].reshape(1, DO)),
            "sels": sels.astype(ml_dtypes.bfloat16),
        })
    return in_maps


def _unshard(results, batch):
    out = np.empty((batch, S, D), dtype=np.float32)
    for b in range(batch):
        out[b, :, 0:DO] = results[2 * b]["out"].astype(np.float32)
        out[b, :, DO:D] = results[2 * b + 1]["out"].astype(np.float32)
    return out


def _run(inputs, trace=False, trace_kwargs=None):
    nc = _get_nc()
    in_maps = _shard(inputs)
    if trace:
        import types
        if "antenv.axon_hooks" not in sys.modules:
            mod = types.ModuleType("antenv.axon_hooks")
            _hook = [None]
            mod.set_axon_ntff_profile_hook = lambda h: _hook.__setitem__(0, h)
            mod.get_axon_ntff_profile_hook = lambda: _hook[0]
            sys.modules["antenv.axon_hooks"] = mod
            from trn_agent_boot.trn_boot import _ntff_profile_via_ctypes
            mod.set_axon_ntff_profile_hook(
                _ntff_profile_via_ctypes("/opt/axon/libaxon_pjrt.so"))
        bass_utils.upload_artifacts = lambda tmpdir: tmpdir
    res = bass_utils.run_bass_kernel_spmd(
        nc, in_maps, core_ids=list(range(8)), trace=trace,
        **(trace_kwargs or {}))
    out = _unshard(res.results, inputs["x"].shape[0])
    return out, res


def kernel(**inputs) -> np.ndarray:
    out, _ = _run(inputs, trace=False)
    return out
